# revision 1
# baseline (speedup 1.0000x reference)
"""Meet-in-the-middle variant: forward DP rows 0..31 + backward DP rows
63..32, combined at the row-31/32 seam. The backward chain is a forward
DP on reversed-row, column-reversed views of the same image tile (2D
negative-stride APs), so both chains run full-width (2-sample-packed,
BIAS-guarded) and the per-op SBUF-access cost is amortized over 128
elements instead of 64. In the backward chain's packed layout, slot 0
holds sample 1 (column-flipped) and slot 1 holds sample 0, so each
sample's seam sum zf+zb carries exactly one +BIAS from each side.
"""

import sys

import numpy as np

sys.path.insert(0, "/opt/trn_rl_repo")

import concourse.bacc as bacc
import concourse.mybir as mybir
import concourse.tile as tile
from concourse.bass_utils import run_bass_kernel_spmd

P = 128
Q = 2
H = 64
W = 64
HH = H // 2      # rows per direction
NB_CORE = P * Q
N_CORES = 8
BIG = 1.0e9
BIAS = 512.0     # slot-0 offset so scan carry can't leak across samples
F32 = mybir.dt.float32
MIN = mybir.AluOpType.min
ADD = mybir.AluOpType.add

_CACHE = {}


def _build():
    nc = bacc.Bacc("TRN2", debug=False, target_bir_lowering=False,
                   num_devices=N_CORES)
    img_d = nc.dram_tensor("images", [NB_CORE, H, W], F32,
                           kind="ExternalInput").ap()
    out_d = nc.dram_tensor("out", [P, Q], F32, kind="ExternalOutput").ap()

    with tile.TileContext(nc) as tc:
        with tc.tile_pool(name="img", bufs=1) as imgp, \
             tc.tile_pool(name="state", bufs=1) as statep, \
             tc.tile_pool(name="work", bufs=4) as workp:
            imgT = imgp.tile([P, H, Q * W], F32)
            zbs = {d: statep.tile([P, Q * W + 1], F32, name=f"zb_{d}")
                   for d in "FB"}
            c0s = {d: statep.tile([P, Q * W], F32, name=f"c0_{d}")
                   for d in "FB"}
            t1 = statep.tile([P, Q * W], F32)
            t2 = statep.tile([P, Q * W], F32)
            red = statep.tile([P, Q], F32)

            def img_row(d, r):
                if d == "F":
                    return imgT[:, r, :]
                return imgT[:, H - 1 - r, ::-1]

            # DMA: alternate chunks from both ends so each chain's next
            # rows arrive just ahead of consumption.
            RC = 4
            for r0 in range(0, HH, RC):
                for a, b in ((r0, r0 + RC), (H - r0 - RC, H - r0)):
                    nc.sync.dma_start(
                        out=imgT[:, a:b, 0:W], in_=img_d[0:P, a:b, :])
                    nc.scalar.dma_start(
                        out=imgT[:, a:b, W:2 * W], in_=img_d[P:2 * P, a:b, :])

            # c0 row-0 scan seed: [-start_node/2 (+BIAS in slot 0), BIG...]
            for d in "FB":
                nc.vector.memset(zbs[d][:, 0:1], BIG)
                nc.vector.memset(c0s[d][:], BIG)
                if d == "F":
                    starts = imgT[:, 0, 0:Q * W:W]          # img[q, 0, 0]
                else:
                    starts = imgT[:, H - 1, Q * W - 1::-W]  # img[1-q, 63, 63]
                nc.vector.tensor_scalar_mul(c0s[d][:, 0:Q * W:W], starts,
                                            -0.5)
                nc.vector.tensor_scalar_add(c0s[d][:, 0:1], c0s[d][:, 0:1],
                                            BIAS)
            for d in "FB":
                nc.vector.tensor_tensor_scan(
                    out=zbs[d][:, 1:], data0=c0s[d][:], data1=img_row(d, 0),
                    initial=BIG, op0=MIN, op1=ADD)

            for r in range(1, HH):
                ms = {}
                for d in "FB":
                    m = workp.tile([P, Q * W], F32, tag=f"m{d}",
                                   name=f"m{d}_{r}")
                    nc.vector.tensor_tensor(out=m[:], in0=zbs[d][:, 1:],
                                            in1=zbs[d][:, 0:Q * W], op=MIN)
                    ms[d] = m
                for d in "FB":
                    nc.vector.tensor_tensor_scan(
                        out=zbs[d][:, 1:], data0=ms[d][:],
                        data1=img_row(d, r), initial=BIG, op0=MIN, op1=ADD)

            # Seam: ans_q = min_j min(zf_q[j]+zb_q[j], zf_q[j]+zb_q[j+1]).
            # zb_q[j] lives at B-slot (1-q), position 63-j -> the doubly
            # reversed view aligns it with zf.
            zf3 = zbs["F"][:, 1:].rearrange("p (q c) -> p q c", q=Q)
            zb3 = zbs["B"][:, 1:].rearrange("p (q c) -> p q c", q=Q)
            zb_rev = zb3[:, ::-1, ::-1]
            t13 = t1[:].rearrange("p (q c) -> p q c", q=Q)
            t23 = t2[:].rearrange("p (q c) -> p q c", q=Q)
            nc.vector.memset(t2[:], BIG)
            nc.vector.tensor_tensor(out=t13[:], in0=zf3, in1=zb_rev, op=ADD)
            nc.vector.tensor_tensor(out=t23[:, :, 0:W - 1],
                                    in0=zf3[:, :, 0:W - 1],
                                    in1=zb_rev[:, :, 1:W], op=ADD)
            nc.vector.tensor_tensor(out=t1[:], in0=t1[:], in1=t2[:], op=MIN)
            nc.vector.tensor_reduce(out=red[:], in_=t13,
                                    axis=mybir.AxisListType.X, op=MIN)
            # each sample's seam sum carries exactly one +BIAS (from F for
            # sample 0, from the B chain's slot 0 for sample 1)
            nc.vector.tensor_scalar_add(red[:], red[:], -BIAS)
            nc.sync.dma_start(out=out_d, in_=red[:])
    nc.compile()
    return nc


def get_nc():
    if "nc" not in _CACHE:
        _CACHE["nc"] = _build()
    return _CACHE["nc"]


def kernel(images: np.ndarray, **run_kwargs) -> np.ndarray:
    B = images.shape[0]
    assert images.shape == (B, H, W) and B == N_CORES * NB_CORE
    images = np.ascontiguousarray(images, dtype=np.float32)
    nc = get_nc()
    in_maps = [{"images": images[c * NB_CORE:(c + 1) * NB_CORE]}
               for c in range(N_CORES)]
    res = run_bass_kernel_spmd(nc, in_maps, core_ids=list(range(N_CORES)),
                               **run_kwargs)
    out = np.empty((B,), dtype=np.float32)
    for c in range(N_CORES):
        out[c * NB_CORE:(c + 1) * NB_CORE] = res.results[c]["out"].T.reshape(-1)
    if run_kwargs:
        return out, res
    return out



# revision 6
# speedup vs baseline: 1.1578x; 1.1578x over previous
"""Meet-in-the-middle DP, fp16 fused-slot variant (DVE-only).

Per core: 256 samples as 128 partitions x 2 sample-pairs. Free-dim layout
per DP row (host-packed, fp16, guard columns = 30000 instead of the old
+512 bias so fp16 keeps full precision on the small DP values):

  [ FS0(64) G FS1(64) G BS1(64) G BS0(64) G ]  = 260 cols

F chains walk rows 0..31; B chains walk rows 63..32 on a both-axes-flipped
view (pre-flipped on the host, so every op is a plain forward scan). Per
row: ONE fused shifted-min TT (fp16, 2x DVE mode) + ONE fused min-add scan
across all four slots. The scan carry is fp32 in hardware regardless of
operand dtype, so fp16 storage only costs one half-ulp requantization per
row (validated ~2e-3 max rel err). Seam at rows 31/32 via fused
tensor_tensor_reduce. Input DMA is host-packed so every chunk is one
contiguous >=520B run per partition (no sub-512B descriptor penalty).
"""

import sys

import numpy as np

sys.path.insert(0, "/opt/trn_rl_repo")

import concourse.bacc as bacc
import concourse.mybir as mybir
import concourse.tile as tile
from concourse.bass_utils import run_bass_kernel_spmd

P = 128
Q = 2
H = 64
W = 64
HH = H // 2
SLOT = W + 1          # 64 data cols + guard
NSLOT = 4
L = NSLOT * SLOT      # 260
NB_CORE = P * Q
N_CORES = 8
GUARD = 30000.0
BIG = 1.0e9
F16 = mybir.dt.float16
F32 = mybir.dt.float32
MIN = mybir.AluOpType.min
ADD = mybir.AluOpType.add

# input DMA chunk boundaries (rows): small first chunks hide startup latency
CHUNKS = (0, 1, 2, 4, 8, 16, 32)

_CACHE = {}


def _build():
    nc = bacc.Bacc("TRN2", debug=False, target_bir_lowering=False,
                   num_devices=N_CORES)
    img_d = nc.dram_tensor("images", [P, HH, L], F16,
                           kind="ExternalInput").ap()
    out_d = nc.dram_tensor("out", [P, Q], F32, kind="ExternalOutput").ap()

    with tile.TileContext(nc) as tc:
        with tc.tile_pool(name="img", bufs=1) as imgp, \
             tc.tile_pool(name="state", bufs=1) as statep, \
             tc.tile_pool(name="work", bufs=2) as workp:
            imgT = imgp.tile([P, HH, L], F16)
            z = statep.tile([P, L + 1], F16)      # [guard | cols 0..259]
            c0 = statep.tile([P, L], F16)
            ta = statep.tile([P, Q, W], F16)      # seam scratch
            tb = statep.tile([P, Q, W - 1], F16)
            r1 = statep.tile([P, Q], F32)
            r2 = statep.tile([P, Q], F32)
            red = statep.tile([P, Q], F32)

            for i, (a, b) in enumerate(zip(CHUNKS[:-1], CHUNKS[1:])):
                eng = nc.sync if i % 2 == 0 else nc.scalar
                eng.dma_start(out=imgT[:, a:b, :], in_=img_d[:, a:b, :])

            # row 0 seed: c0 = GUARD except -img_start/2 at slot starts
            nc.vector.memset(z[:, 0:1], GUARD)
            nc.vector.memset(c0[:], GUARD)
            nc.vector.tensor_scalar_mul(c0[:, 0:L:SLOT],
                                        imgT[:, 0, 0:L:SLOT], -0.5)
            nc.vector.tensor_tensor_scan(
                out=z[:, 1:], data0=c0[:], data1=imgT[:, 0, :],
                initial=BIG, op0=MIN, op1=ADD)

            for r in range(1, HH):
                m = workp.tile([P, L], F16, tag="m", name=f"m_{r}")
                nc.vector.tensor_tensor(out=m[:], in0=z[:, 1:],
                                        in1=z[:, 0:L], op=MIN)
                nc.vector.tensor_tensor_scan(
                    out=z[:, 1:], data0=m[:], data1=imgT[:, r, :],
                    initial=BIG, op0=MIN, op1=ADD)

            # seam at rows 31/32: ans = min_j min(zf[j]+zb[j], zf[j]+zb[j+1])
            # zb stored col-flipped -> read reversed to align orig columns.
            # sample q's F chain is slot q, its B chain slot 3-q.
            y3 = z[:, 1:].rearrange("p (s c) -> p s c", s=NSLOT)
            zf3 = y3[:, 0:Q, 0:W]
            zbr3 = y3[:, NSLOT - 1:1:-1, W - 1::-1]
            nc.vector.tensor_tensor(out=ta[:], in0=zf3, in1=zbr3, op=ADD)
            nc.vector.tensor_tensor(out=tb[:], in0=zf3[:, :, 0:W - 1],
                                    in1=zbr3[:, :, 1:W], op=ADD)
            nc.vector.tensor_reduce(out=r1[:], in_=ta[:],
                                    axis=mybir.AxisListType.X, op=MIN)
            nc.vector.tensor_reduce(out=r2[:], in_=tb[:],
                                    axis=mybir.AxisListType.X, op=MIN)
            nc.vector.tensor_tensor(out=red[:], in0=r1[:], in1=r2[:], op=MIN)
            nc.sync.dma_start(out=out_d, in_=red[:])
    nc.compile()
    return nc


def get_nc():
    if "nc" not in _CACHE:
        _CACHE["nc"] = _build()
    return _CACHE["nc"]


def _pack(images):
    """[2048,64,64] f32 -> per-core [8][128, 32, 260] f16 host-side."""
    img16 = np.ascontiguousarray(images, dtype=np.float16)
    blocks = img16.reshape(N_CORES, Q, P, H, W)
    out = np.full((N_CORES, P, HH, L), GUARD, dtype=np.float16)
    for c in range(N_CORES):
        s0 = blocks[c, 0]
        s1 = blocks[c, 1]
        out[c, :, :, 0 * SLOT:0 * SLOT + W] = s0[:, :HH, :]
        out[c, :, :, 1 * SLOT:1 * SLOT + W] = s1[:, :HH, :]
        out[c, :, :, 2 * SLOT:2 * SLOT + W] = np.flip(s1, axis=(1, 2))[:, :HH, :]
        out[c, :, :, 3 * SLOT:3 * SLOT + W] = np.flip(s0, axis=(1, 2))[:, :HH, :]
    return out


def kernel(images: np.ndarray, **run_kwargs) -> np.ndarray:
    B = images.shape[0]
    assert images.shape == (B, H, W) and B == N_CORES * NB_CORE
    packed = _pack(images)
    nc = get_nc()
    in_maps = [{"images": packed[c]} for c in range(N_CORES)]
    res = run_bass_kernel_spmd(nc, in_maps, core_ids=list(range(N_CORES)),
                               **run_kwargs)
    out = np.empty((B,), dtype=np.float32)
    for c in range(N_CORES):
        out[c * NB_CORE:(c + 1) * NB_CORE] = res.results[c]["out"].T.reshape(-1)
    if run_kwargs:
        return out, res
    return out


# revision 9
# speedup vs baseline: 1.2508x; 1.0804x over previous
"""Meet-in-the-middle DP, fp16 two-chain interleaved variant.

Per core: 256 samples as 128 partitions x 2 sample-pairs. Free-dim layout
per DP row (host-packed, fp16, guard columns = 30000 instead of a +512
bias so fp16 keeps full precision on the small DP values):

  [ FS0(64) G FS1(64) G | BS1(64) G BS0(64) G ]  = 260 cols
    chain A (130)          chain B (130)

F chains walk rows 0..31; B chains walk rows 63..32 on a both-axes-flipped
view (pre-flipped on the host, so every op is a plain forward scan). Per
row, chain A and chain B each run one shifted-min TT (fp16, 2x DVE mode)
and one min-add scan, interleaved TT_A,TT_B,scan_A,scan_B so each
instruction's semaphore wait (95ns update latency) is hidden under the
other chain's execution — the DVE engine runs gap-free. The scan carry is
fp32 in hardware regardless of operand dtype, so fp16 storage only costs
one half-ulp requantization per row (validated ~2e-3 max rel err).
Input DMA is host-packed so every chunk is one contiguous >=520B run per
partition (no sub-512B descriptor penalty).
"""

import sys

import numpy as np

sys.path.insert(0, "/opt/trn_rl_repo")

import concourse.bacc as bacc
import concourse.mybir as mybir
import concourse.tile as tile
from concourse.bass_utils import run_bass_kernel_spmd

P = 128
Q = 2
H = 64
W = 64
HH = H // 2
SLOT = W + 1          # 64 data cols + guard
NSLOT = 4
L = NSLOT * SLOT      # 260
CW = 2 * SLOT         # 130: per-chain width
NB_CORE = P * Q
N_CORES = 8
GUARD = 30000.0
BIG = 1.0e9
F16 = mybir.dt.float16
F32 = mybir.dt.float32
MIN = mybir.AluOpType.min
ADD = mybir.AluOpType.add

# input DMA chunk boundaries (rows): small first chunks hide startup latency
CHUNKS = (0, 1, 2, 4, 8, 16, 32)

_CACHE = {}


def _build():
    nc = bacc.Bacc("TRN2", debug=False, target_bir_lowering=False,
                   num_devices=N_CORES)
    img_d = nc.dram_tensor("images", [P, HH, L], F16,
                           kind="ExternalInput").ap()
    out_d = nc.dram_tensor("out", [P, Q], F32, kind="ExternalOutput").ap()

    with tile.TileContext(nc) as tc:
        with tc.tile_pool(name="img", bufs=1) as imgp, \
             tc.tile_pool(name="state", bufs=1) as statep, \
             tc.tile_pool(name="work", bufs=2) as workp:
            imgT = imgp.tile([P, HH, L], F16)
            za = statep.tile([P, CW + 1], F16)    # [guard | cols 0..129]
            zb = statep.tile([P, CW + 1], F16)    # [guard | cols 130..259]
            c0a = statep.tile([P, CW], F16)
            c0b = statep.tile([P, CW], F16)
            tt = statep.tile([P, Q, 2 * W - 1], F16)   # seam scratch
            red = statep.tile([P, Q], F32)

            # no-dependency prep first: runs during the DMA fill latency
            nc.vector.memset(za[:, 0:1], GUARD)
            nc.vector.memset(zb[:, 0:1], GUARD)
            nc.vector.memset(c0a[:], GUARD)
            nc.vector.memset(c0b[:], GUARD)

            for i, (a, b) in enumerate(zip(CHUNKS[:-1], CHUNKS[1:])):
                eng = nc.sync if i % 2 == 0 else nc.scalar
                eng.dma_start(out=imgT[:, a:b, :], in_=img_d[:, a:b, :])

            # row 0 seed: c0 = GUARD except -img_start/2 at slot starts
            nc.vector.tensor_scalar_mul(c0a[:, 0:CW:SLOT],
                                        imgT[:, 0, 0:CW:SLOT], -0.5)
            nc.vector.tensor_scalar_mul(c0b[:, 0:CW:SLOT],
                                        imgT[:, 0, CW:L:SLOT], -0.5)
            nc.vector.tensor_tensor_scan(
                out=za[:, 1:], data0=c0a[:], data1=imgT[:, 0, 0:CW],
                initial=BIG, op0=MIN, op1=ADD)
            nc.vector.tensor_tensor_scan(
                out=zb[:, 1:], data0=c0b[:], data1=imgT[:, 0, CW:L],
                initial=BIG, op0=MIN, op1=ADD)

            for r in range(1, HH):
                ma = workp.tile([P, CW], F16, tag="ma", name=f"ma_{r}")
                mb = workp.tile([P, CW], F16, tag="mb", name=f"mb_{r}")
                nc.vector.tensor_tensor(out=ma[:], in0=za[:, 1:],
                                        in1=za[:, 0:CW], op=MIN)
                nc.vector.tensor_tensor(out=mb[:], in0=zb[:, 1:],
                                        in1=zb[:, 0:CW], op=MIN)
                nc.vector.tensor_tensor_scan(
                    out=za[:, 1:], data0=ma[:], data1=imgT[:, r, 0:CW],
                    initial=BIG, op0=MIN, op1=ADD)
                nc.vector.tensor_tensor_scan(
                    out=zb[:, 1:], data0=mb[:], data1=imgT[:, r, CW:L],
                    initial=BIG, op0=MIN, op1=ADD)

            # seam at rows 31/32: ans = min_j min(zf[j]+zb[j], zf[j]+zb[j+1])
            # B chains stored col-flipped; sample q's B chain is slot 1-q of
            # zb, so reverse both slot and col axes to align original cols.
            zf3 = za[:, 1:].rearrange("p (s c) -> p s c", s=2)[:, :, 0:W]
            zbr3 = zb[:, 1:].rearrange("p (s c) -> p s c", s=2)[:, ::-1,
                                                               W - 1::-1]
            nc.vector.tensor_tensor(out=tt[:, :, 0:W], in0=zf3, in1=zbr3,
                                    op=ADD)
            nc.vector.tensor_tensor(out=tt[:, :, W:2 * W - 1],
                                    in0=zf3[:, :, 0:W - 1],
                                    in1=zbr3[:, :, 1:W], op=ADD)
            nc.vector.tensor_reduce(out=red[:], in_=tt[:],
                                    axis=mybir.AxisListType.X, op=MIN)
            nc.sync.dma_start(out=out_d, in_=red[:])
    nc.compile()
    return nc


def get_nc():
    if "nc" not in _CACHE:
        _CACHE["nc"] = _build()
    return _CACHE["nc"]


def _pack(images):
    """[2048,64,64] f32 -> per-core [8][128, 32, 260] f16 host-side."""
    img16 = np.ascontiguousarray(images, dtype=np.float16)
    blocks = img16.reshape(N_CORES, Q, P, H, W)
    out = np.full((N_CORES, P, HH, L), GUARD, dtype=np.float16)
    for c in range(N_CORES):
        s0 = blocks[c, 0]
        s1 = blocks[c, 1]
        out[c, :, :, 0 * SLOT:0 * SLOT + W] = s0[:, :HH, :]
        out[c, :, :, 1 * SLOT:1 * SLOT + W] = s1[:, :HH, :]
        out[c, :, :, 2 * SLOT:2 * SLOT + W] = np.flip(s1, axis=(1, 2))[:, :HH, :]
        out[c, :, :, 3 * SLOT:3 * SLOT + W] = np.flip(s0, axis=(1, 2))[:, :HH, :]
    return out


def kernel(images: np.ndarray, **run_kwargs) -> np.ndarray:
    B = images.shape[0]
    assert images.shape == (B, H, W) and B == N_CORES * NB_CORE
    packed = _pack(images)
    nc = get_nc()
    in_maps = [{"images": packed[c]} for c in range(N_CORES)]
    res = run_bass_kernel_spmd(nc, in_maps, core_ids=list(range(N_CORES)),
                               **run_kwargs)
    out = np.empty((B,), dtype=np.float32)
    for c in range(N_CORES):
        out[c * NB_CORE:(c + 1) * NB_CORE] = res.results[c]["out"].T.reshape(-1)
    if run_kwargs:
        return out, res
    return out


# revision 12
# speedup vs baseline: 1.2538x; 1.0023x over previous
"""Meet-in-the-middle DP, fp16 two-chain interleaved variant.

Per core: 256 samples as 128 partitions x 2 sample-pairs. Free-dim layout
per DP row (host-packed, fp16, guard columns = 30000 instead of a +512
bias so fp16 keeps full precision on the small DP values):

  [ FS0(64) G FS1(64) G | BS1(64) G BS0(64) G ]  = 260 cols
    chain A (130)          chain B (130)

F chains walk rows 0..31; B chains walk rows 63..32 on a both-axes-flipped
view (pre-flipped on the host, so every op is a plain forward scan). Per
row, chain A and chain B each run one shifted-min TT (fp16, 2x DVE mode)
and one min-add scan, interleaved TT_A,TT_B,scan_A,scan_B so each
instruction's semaphore wait (95ns update latency) is hidden under the
other chain's execution — the DVE engine runs gap-free. The scan carry is
fp32 in hardware regardless of operand dtype, so fp16 storage only costs
one half-ulp requantization per row (validated ~2e-3 max rel err).
Input DMA is host-packed so every chunk is one contiguous >=520B run per
partition (no sub-512B descriptor penalty).
"""

import sys

import numpy as np

sys.path.insert(0, "/opt/trn_rl_repo")

import concourse.bacc as bacc
import concourse.mybir as mybir
import concourse.tile as tile
from concourse.bass_utils import run_bass_kernel_spmd

P = 128
Q = 2
H = 64
W = 64
HH = H // 2
SLOT = W + 1          # 64 data cols + guard
NSLOT = 4
L = NSLOT * SLOT      # 260
CW = 2 * SLOT         # 130: per-chain layout width
AW = CW - 1           # 129: active op width (trailing guard col never read)
NB_CORE = P * Q
N_CORES = 8
GUARD = 30000.0
BIG = 1.0e9
F16 = mybir.dt.float16
F32 = mybir.dt.float32
MIN = mybir.AluOpType.min
ADD = mybir.AluOpType.add

# input DMA chunk boundaries (rows): small first chunks hide startup latency
CHUNKS = (0, 1, 2, 4, 8, 16, 32)

_CACHE = {}


def _build():
    nc = bacc.Bacc("TRN2", debug=False, target_bir_lowering=False,
                   num_devices=N_CORES)
    img_d = nc.dram_tensor("images", [P, HH, L], F16,
                           kind="ExternalInput").ap()
    out_d = nc.dram_tensor("out", [P, Q], F32, kind="ExternalOutput").ap()

    with tile.TileContext(nc) as tc:
        with tc.tile_pool(name="img", bufs=1) as imgp, \
             tc.tile_pool(name="state", bufs=1) as statep, \
             tc.tile_pool(name="work", bufs=2) as workp:
            imgT = imgp.tile([P, HH, L], F16)
            za = statep.tile([P, CW + 1], F16)    # [guard | cols 0..129]
            zb = statep.tile([P, CW + 1], F16)    # [guard | cols 130..259]
            c0a = statep.tile([P, AW], F16)
            c0b = statep.tile([P, AW], F16)
            tt = statep.tile([P, Q, 2 * W - 1], F16)   # seam scratch
            red = statep.tile([P, Q], F32)

            # no-dependency prep first: runs during the DMA fill latency
            nc.vector.memset(za[:, 0:1], GUARD)
            nc.vector.memset(zb[:, 0:1], GUARD)
            nc.vector.memset(c0a[:], GUARD)
            nc.vector.memset(c0b[:], GUARD)

            for i, (a, b) in enumerate(zip(CHUNKS[:-1], CHUNKS[1:])):
                eng = nc.sync if i % 2 == 0 else nc.scalar
                eng.dma_start(out=imgT[:, a:b, :], in_=img_d[:, a:b, :])

            # row 0 seed: c0 = GUARD except -img_start/2 at slot starts
            nc.vector.tensor_scalar_mul(c0a[:, 0:AW:SLOT],
                                        imgT[:, 0, 0:AW:SLOT], -0.5)
            nc.vector.tensor_scalar_mul(c0b[:, 0:AW:SLOT],
                                        imgT[:, 0, CW:CW + AW:SLOT], -0.5)
            nc.vector.tensor_tensor_scan(
                out=za[:, 1:1 + AW], data0=c0a[:], data1=imgT[:, 0, 0:AW],
                initial=BIG, op0=MIN, op1=ADD)
            nc.vector.tensor_tensor_scan(
                out=zb[:, 1:1 + AW], data0=c0b[:],
                data1=imgT[:, 0, CW:CW + AW],
                initial=BIG, op0=MIN, op1=ADD)

            for r in range(1, HH):
                ma = workp.tile([P, AW], F16, tag="ma", name=f"ma_{r}")
                mb = workp.tile([P, AW], F16, tag="mb", name=f"mb_{r}")
                nc.vector.tensor_tensor(out=ma[:], in0=za[:, 1:1 + AW],
                                        in1=za[:, 0:AW], op=MIN)
                nc.vector.tensor_tensor(out=mb[:], in0=zb[:, 1:1 + AW],
                                        in1=zb[:, 0:AW], op=MIN)
                nc.vector.tensor_tensor_scan(
                    out=za[:, 1:1 + AW], data0=ma[:], data1=imgT[:, r, 0:AW],
                    initial=BIG, op0=MIN, op1=ADD)
                nc.vector.tensor_tensor_scan(
                    out=zb[:, 1:1 + AW], data0=mb[:],
                    data1=imgT[:, r, CW:CW + AW],
                    initial=BIG, op0=MIN, op1=ADD)

            # seam at rows 31/32: ans = min_j min(zf[j]+zb[j], zf[j]+zb[j+1])
            # B chains stored col-flipped; sample q's B chain is slot 1-q of
            # zb, so reverse both slot and col axes to align original cols.
            zf3 = za[:, 1:].rearrange("p (s c) -> p s c", s=2)[:, :, 0:W]
            zbr3 = zb[:, 1:].rearrange("p (s c) -> p s c", s=2)[:, ::-1,
                                                               W - 1::-1]
            nc.vector.tensor_tensor(out=tt[:, :, 0:W], in0=zf3, in1=zbr3,
                                    op=ADD)
            nc.vector.tensor_tensor(out=tt[:, :, W:2 * W - 1],
                                    in0=zf3[:, :, 0:W - 1],
                                    in1=zbr3[:, :, 1:W], op=ADD)
            nc.vector.tensor_reduce(out=red[:], in_=tt[:],
                                    axis=mybir.AxisListType.X, op=MIN)
            nc.sync.dma_start(out=out_d, in_=red[:])
    nc.compile()
    return nc


def get_nc():
    if "nc" not in _CACHE:
        _CACHE["nc"] = _build()
    return _CACHE["nc"]


def _pack(images):
    """[2048,64,64] f32 -> per-core [8][128, 32, 260] f16 host-side."""
    img16 = np.ascontiguousarray(images, dtype=np.float16)
    blocks = img16.reshape(N_CORES, Q, P, H, W)
    out = np.full((N_CORES, P, HH, L), GUARD, dtype=np.float16)
    for c in range(N_CORES):
        s0 = blocks[c, 0]
        s1 = blocks[c, 1]
        out[c, :, :, 0 * SLOT:0 * SLOT + W] = s0[:, :HH, :]
        out[c, :, :, 1 * SLOT:1 * SLOT + W] = s1[:, :HH, :]
        out[c, :, :, 2 * SLOT:2 * SLOT + W] = np.flip(s1, axis=(1, 2))[:, :HH, :]
        out[c, :, :, 3 * SLOT:3 * SLOT + W] = np.flip(s0, axis=(1, 2))[:, :HH, :]
    return out


def kernel(images: np.ndarray, **run_kwargs) -> np.ndarray:
    B = images.shape[0]
    assert images.shape == (B, H, W) and B == N_CORES * NB_CORE
    packed = _pack(images)
    nc = get_nc()
    in_maps = [{"images": packed[c]} for c in range(N_CORES)]
    res = run_bass_kernel_spmd(nc, in_maps, core_ids=list(range(N_CORES)),
                               **run_kwargs)
    out = np.empty((B,), dtype=np.float32)
    for c in range(N_CORES):
        out[c * NB_CORE:(c + 1) * NB_CORE] = res.results[c]["out"].T.reshape(-1)
    if run_kwargs:
        return out, res
    return out


# revision 15
# speedup vs baseline: 1.2556x; 1.0015x over previous
"""Meet-in-the-middle DP, fp16 two-chain interleaved variant.

Per core: 256 samples as 128 partitions x 2 sample-pairs. Free-dim layout
per DP row (host-packed, fp16, guard columns = 30000 instead of a +512
bias so fp16 keeps full precision on the small DP values):

  [ FS0(64) G FS1(64) G | BS1(64) G BS0(64) G ]  = 260 cols
    chain A (130)          chain B (130)

F chains walk rows 0..31; B chains walk rows 63..32 on a both-axes-flipped
view (pre-flipped on the host, so every op is a plain forward scan). Per
row, chain A and chain B each run one shifted-min TT (fp16, 2x DVE mode)
and one min-add scan, interleaved TT_A,TT_B,scan_A,scan_B so each
instruction's semaphore wait (95ns update latency) is hidden under the
other chain's execution — the DVE engine runs gap-free. The scan carry is
fp32 in hardware regardless of operand dtype, so fp16 storage only costs
one half-ulp requantization per row (validated ~2e-3 max rel err).
Input DMA is host-packed so every chunk is one contiguous >=520B run per
partition (no sub-512B descriptor penalty).
"""

import sys

import numpy as np

sys.path.insert(0, "/opt/trn_rl_repo")

import concourse.bacc as bacc
import concourse.mybir as mybir
import concourse.tile as tile
from concourse.bass_utils import run_bass_kernel_spmd

P = 128
Q = 2
H = 64
W = 64
HH = H // 2
SLOT = W + 1          # 64 data cols + guard
NSLOT = 4
L = NSLOT * SLOT      # 260
CW = 2 * SLOT         # 130: per-chain layout width
AW = CW - 1           # 129: active op width (trailing guard col never read)
NB_CORE = P * Q
N_CORES = 8
GUARD = 30000.0
BIG = 1.0e9
F16 = mybir.dt.float16
F32 = mybir.dt.float32
MIN = mybir.AluOpType.min
ADD = mybir.AluOpType.add

# input DMA chunk boundaries (rows): small first chunks hide startup latency
CHUNKS = (0, 1, 2, 4, 8, 16, 32)

_CACHE = {}


def _build():
    nc = bacc.Bacc("TRN2", debug=False, target_bir_lowering=False,
                   num_devices=N_CORES)
    img_d = nc.dram_tensor("images", [P, HH, L], F16,
                           kind="ExternalInput").ap()
    out_d = nc.dram_tensor("out", [P, Q], F32, kind="ExternalOutput").ap()

    with tile.TileContext(nc) as tc:
        with tc.tile_pool(name="img", bufs=1) as imgp, \
             tc.tile_pool(name="state", bufs=1) as statep, \
             tc.tile_pool(name="work", bufs=2) as workp:
            imgT = imgp.tile([P, HH, L], F16)
            za = statep.tile([P, CW + 1], F16)    # [guard | cols 0..129]
            zb = statep.tile([P, CW + 1], F16)    # [guard | cols 130..259]
            c0a = statep.tile([P, AW], F16)
            c0b = statep.tile([P, AW], F16)
            cc = statep.tile([P, Q, W], F16)      # seam scratch
            t2 = statep.tile([P, Q, W], F16)
            red = statep.tile([P, Q], F32)

            # no-dependency prep first: runs during the DMA fill latency
            nc.vector.memset(za[:, 0:1], GUARD)
            nc.vector.memset(zb[:, 0:1], GUARD)
            nc.vector.memset(c0a[:], GUARD)
            nc.vector.memset(c0b[:], GUARD)

            for i, (a, b) in enumerate(zip(CHUNKS[:-1], CHUNKS[1:])):
                eng = nc.sync if i % 2 == 0 else nc.scalar
                eng.dma_start(out=imgT[:, a:b, :], in_=img_d[:, a:b, :])

            # row 0 seed: c0 = GUARD except -img_start/2 at slot starts
            nc.vector.tensor_scalar_mul(c0a[:, 0:AW:SLOT],
                                        imgT[:, 0, 0:AW:SLOT], -0.5)
            nc.vector.tensor_scalar_mul(c0b[:, 0:AW:SLOT],
                                        imgT[:, 0, CW:CW + AW:SLOT], -0.5)
            nc.vector.tensor_tensor_scan(
                out=za[:, 1:1 + AW], data0=c0a[:], data1=imgT[:, 0, 0:AW],
                initial=BIG, op0=MIN, op1=ADD)
            nc.vector.tensor_tensor_scan(
                out=zb[:, 1:1 + AW], data0=c0b[:],
                data1=imgT[:, 0, CW:CW + AW],
                initial=BIG, op0=MIN, op1=ADD)

            for r in range(1, HH):
                ma = workp.tile([P, AW], F16, tag="ma", name=f"ma_{r}")
                mb = workp.tile([P, AW], F16, tag="mb", name=f"mb_{r}")
                # last row: chain B first throughout, so the seam's zb-only
                # ops start right after scan_B, hidden under chain A's scan
                chains = [
                    (za, ma, imgT[:, r, 0:AW]),
                    (zb, mb, imgT[:, r, CW:CW + AW]),
                ]
                if r == HH - 1:
                    chains = chains[::-1]
                for z_, m_, im_ in chains:
                    nc.vector.tensor_tensor(out=m_[:], in0=z_[:, 1:1 + AW],
                                            in1=z_[:, 0:AW], op=MIN)
                for z_, m_, im_ in chains:
                    nc.vector.tensor_tensor_scan(
                        out=z_[:, 1:1 + AW], data0=m_[:], data1=im_,
                        initial=BIG, op0=MIN, op1=ADD)

            # seam at rows 31/32: ans_q = min_j (zf_q[j] + c_q[j]) with
            # c_q[j] = min(zb_q[j], zb_q[j+1]) (down / diag edges).
            # B chains stored col-flipped; sample q's B chain is slot 1-q of
            # zb, so read zb reversed; index j+1 = one col further left, and
            # the j=63 diag candidate reads a guard (no such edge) - exact.
            # Per-sample ops so every dependency is >=2 instructions back:
            # the DVE runs the whole seam gap-free.
            zbx = (zb[:, CW - 1::-1], zb[:, SLOT - 1::-1])   # zb_q[j..], 65 wide
            zfx = (za[:, 1:1 + W], za[:, 1 + SLOT:1 + SLOT + W])
            for q in range(Q):
                nc.vector.tensor_tensor(out=cc[:, q, :], in0=zbx[q][:, 0:W],
                                        in1=zbx[q][:, 1:W + 1], op=MIN)
            for q in range(Q):
                nc.vector.tensor_tensor(out=t2[:, q, :], in0=zfx[q],
                                        in1=cc[:, q, :], op=ADD)
            for q in range(Q):
                nc.vector.tensor_reduce(out=red[:, q:q + 1], in_=t2[:, q, :],
                                        axis=mybir.AxisListType.X, op=MIN)
            nc.sync.dma_start(out=out_d, in_=red[:])
    nc.compile()
    return nc


def get_nc():
    if "nc" not in _CACHE:
        _CACHE["nc"] = _build()
    return _CACHE["nc"]


def _pack(images):
    """[2048,64,64] f32 -> per-core [8][128, 32, 260] f16 host-side."""
    img16 = np.ascontiguousarray(images, dtype=np.float16)
    blocks = img16.reshape(N_CORES, Q, P, H, W)
    out = np.full((N_CORES, P, HH, L), GUARD, dtype=np.float16)
    for c in range(N_CORES):
        s0 = blocks[c, 0]
        s1 = blocks[c, 1]
        out[c, :, :, 0 * SLOT:0 * SLOT + W] = s0[:, :HH, :]
        out[c, :, :, 1 * SLOT:1 * SLOT + W] = s1[:, :HH, :]
        out[c, :, :, 2 * SLOT:2 * SLOT + W] = np.flip(s1, axis=(1, 2))[:, :HH, :]
        out[c, :, :, 3 * SLOT:3 * SLOT + W] = np.flip(s0, axis=(1, 2))[:, :HH, :]
    return out


def kernel(images: np.ndarray, **run_kwargs) -> np.ndarray:
    B = images.shape[0]
    assert images.shape == (B, H, W) and B == N_CORES * NB_CORE
    packed = _pack(images)
    nc = get_nc()
    in_maps = [{"images": packed[c]} for c in range(N_CORES)]
    res = run_bass_kernel_spmd(nc, in_maps, core_ids=list(range(N_CORES)),
                               **run_kwargs)
    out = np.empty((B,), dtype=np.float32)
    for c in range(N_CORES):
        out[c * NB_CORE:(c + 1) * NB_CORE] = res.results[c]["out"].T.reshape(-1)
    if run_kwargs:
        return out, res
    return out


# revision 17
# speedup vs baseline: 1.2600x; 1.0035x over previous
"""Meet-in-the-middle DP, fp16 two-chain interleaved variant.

Per core: 256 samples as 128 partitions x 2 sample-pairs. Free-dim layout
per DP row (host-packed, fp16, guard columns = 30000 instead of a +512
bias so fp16 keeps full precision on the small DP values):

  [ FS0(64) G FS1(64) G | BS1(64) G BS0(64) G ]  = 260 cols
    chain A (130)          chain B (130)

F chains walk rows 0..31; B chains walk rows 63..32 on a both-axes-flipped
view (pre-flipped on the host, so every op is a plain forward scan). Per
row, chain A and chain B each run one shifted-min TT (fp16, 2x DVE mode)
and one min-add scan, interleaved TT_A,TT_B,scan_A,scan_B so each
instruction's semaphore wait (95ns update latency) is hidden under the
other chain's execution — the DVE engine runs gap-free. The scan carry is
fp32 in hardware regardless of operand dtype, so fp16 storage only costs
one half-ulp requantization per row (validated ~2e-3 max rel err).
Input DMA is host-packed so every chunk is one contiguous >=520B run per
partition (no sub-512B descriptor penalty).
"""

import sys

import numpy as np

sys.path.insert(0, "/opt/trn_rl_repo")

import concourse.bacc as bacc
import concourse.mybir as mybir
import concourse.tile as tile
from concourse.bass_utils import run_bass_kernel_spmd

P = 128
Q = 2
H = 64
W = 64
HH = H // 2
SLOT = W + 1          # 64 data cols + guard
NSLOT = 4
L = NSLOT * SLOT      # 260
CW = 2 * SLOT         # 130: per-chain layout width
AW = CW - 1           # 129: active op width (trailing guard col never read)
NB_CORE = P * Q
N_CORES = 8
GUARD = 30000.0
BIG = 1.0e9
F16 = mybir.dt.float16
F32 = mybir.dt.float32
MIN = mybir.AluOpType.min
ADD = mybir.AluOpType.add

# input DMA chunk boundaries (rows): small first chunks hide startup latency
CHUNKS = (0, 1, 2, 4, 8, 16, 32)

_CACHE = {}


def _build():
    nc = bacc.Bacc("TRN2", debug=False, target_bir_lowering=False,
                   num_devices=N_CORES)
    img_d = nc.dram_tensor("images", [P, HH, L], F16,
                           kind="ExternalInput").ap()
    out_d = nc.dram_tensor("out", [P, Q], F32, kind="ExternalOutput").ap()

    with tile.TileContext(nc) as tc:
        with tc.tile_pool(name="img", bufs=1) as imgp, \
             tc.tile_pool(name="state", bufs=1) as statep, \
             tc.tile_pool(name="work", bufs=2) as workp:
            imgT = imgp.tile([P, HH, L], F16)
            za = statep.tile([P, CW + 1], F16)    # [guard | cols 0..129]
            zb = statep.tile([P, CW + 1], F16)    # [guard | cols 130..259]
            c0a = statep.tile([P, AW], F16)
            c0b = statep.tile([P, AW], F16)
            cc = statep.tile([P, Q, W], F16)      # seam scratch
            t2 = statep.tile([P, Q, W], F16)
            red = statep.tile([P, Q], F32)

            # no-dependency prep first: runs during the DMA fill latency
            nc.vector.memset(za[:, 0:1], GUARD)
            nc.vector.memset(zb[:, 0:1], GUARD)
            nc.vector.memset(c0a[:], GUARD)
            nc.vector.memset(c0b[:], GUARD)

            for i, (a, b) in enumerate(zip(CHUNKS[:-1], CHUNKS[1:])):
                eng = nc.sync if i % 2 == 0 else nc.scalar
                eng.dma_start(out=imgT[:, a:b, :], in_=img_d[:, a:b, :])

            # row 0 seed: c0 = GUARD except -img_start/2 at slot starts
            nc.vector.tensor_scalar_mul(c0a[:, 0:AW:SLOT],
                                        imgT[:, 0, 0:AW:SLOT], -0.5)
            nc.vector.tensor_scalar_mul(c0b[:, 0:AW:SLOT],
                                        imgT[:, 0, CW:CW + AW:SLOT], -0.5)
            nc.vector.tensor_tensor_scan(
                out=za[:, 1:1 + AW], data0=c0a[:], data1=imgT[:, 0, 0:AW],
                initial=BIG, op0=MIN, op1=ADD)
            nc.vector.tensor_tensor_scan(
                out=zb[:, 1:1 + AW], data0=c0b[:],
                data1=imgT[:, 0, CW:CW + AW],
                initial=BIG, op0=MIN, op1=ADD)

            for r in range(1, HH):
                ma = workp.tile([P, AW], F16, tag="ma", name=f"ma_{r}")
                mb = workp.tile([P, AW], F16, tag="mb", name=f"mb_{r}")
                nc.vector.tensor_tensor(out=ma[:], in0=za[:, 1:1 + AW],
                                        in1=za[:, 0:AW], op=MIN)
                nc.vector.tensor_tensor(out=mb[:], in0=zb[:, 1:1 + AW],
                                        in1=zb[:, 0:AW], op=MIN)
                nc.vector.tensor_tensor_scan(
                    out=za[:, 1:1 + AW], data0=ma[:], data1=imgT[:, r, 0:AW],
                    initial=BIG, op0=MIN, op1=ADD)
                nc.vector.tensor_tensor_scan(
                    out=zb[:, 1:1 + AW], data0=mb[:],
                    data1=imgT[:, r, CW:CW + AW],
                    initial=BIG, op0=MIN, op1=ADD)

            # seam at rows 31/32, grouped by the row-32 column k:
            #   ans_q = min_k ( zb_q[k] + min(zf_q[k], zf_q[k-1]) )
            # (down edge k->k, diag edge k-1->k; zf[-1] reads a guard, so the
            # nonexistent edge never wins). The cc ops read only chain A,
            # which the scheduler sequences first, so every seam dependency
            # is >=2 instructions back and the DVE runs the seam gap-free.
            # B chains stored col-flipped (sample q = slot 1-q, reversed).
            zfs = (za[:, 1:1 + W], za[:, 1 + SLOT:1 + SLOT + W])
            zfp = (za[:, 0:W], za[:, SLOT:SLOT + W])         # zf[k-1] views
            zbr = (zb[:, CW - 1:SLOT:-1], zb[:, SLOT - 1:0:-1])
            for q in range(Q):
                nc.vector.tensor_tensor(out=cc[:, q, :], in0=zfs[q],
                                        in1=zfp[q], op=MIN)
            for q in range(Q):
                nc.vector.tensor_tensor(out=t2[:, q, :], in0=cc[:, q, :],
                                        in1=zbr[q], op=ADD)
            for q in range(Q):
                nc.vector.tensor_reduce(out=red[:, q:q + 1], in_=t2[:, q, :],
                                        axis=mybir.AxisListType.X, op=MIN)
            nc.sync.dma_start(out=out_d, in_=red[:])
    nc.compile()
    return nc


def get_nc():
    if "nc" not in _CACHE:
        _CACHE["nc"] = _build()
    return _CACHE["nc"]


def _pack(images):
    """[2048,64,64] f32 -> per-core [8][128, 32, 260] f16 host-side."""
    img16 = np.ascontiguousarray(images, dtype=np.float16)
    blocks = img16.reshape(N_CORES, Q, P, H, W)
    out = np.full((N_CORES, P, HH, L), GUARD, dtype=np.float16)
    for c in range(N_CORES):
        s0 = blocks[c, 0]
        s1 = blocks[c, 1]
        out[c, :, :, 0 * SLOT:0 * SLOT + W] = s0[:, :HH, :]
        out[c, :, :, 1 * SLOT:1 * SLOT + W] = s1[:, :HH, :]
        out[c, :, :, 2 * SLOT:2 * SLOT + W] = np.flip(s1, axis=(1, 2))[:, :HH, :]
        out[c, :, :, 3 * SLOT:3 * SLOT + W] = np.flip(s0, axis=(1, 2))[:, :HH, :]
    return out


def kernel(images: np.ndarray, **run_kwargs) -> np.ndarray:
    B = images.shape[0]
    assert images.shape == (B, H, W) and B == N_CORES * NB_CORE
    packed = _pack(images)
    nc = get_nc()
    in_maps = [{"images": packed[c]} for c in range(N_CORES)]
    res = run_bass_kernel_spmd(nc, in_maps, core_ids=list(range(N_CORES)),
                               **run_kwargs)
    out = np.empty((B,), dtype=np.float32)
    for c in range(N_CORES):
        out[c * NB_CORE:(c + 1) * NB_CORE] = res.results[c]["out"].T.reshape(-1)
    if run_kwargs:
        return out, res
    return out
